# revision 5
# baseline (speedup 1.0000x reference)
"""Causal single-head attention (B=4, S=2048, d=1024, f32) on 8 TRN2 NeuronCores.

Sharding: core i = (batch b = i//2, half h = i%2); no collectives. Each core
computes the QKV projections for its batch (K,V over all 2048 rows, Q over its
1024 query rows) and causal attention for its 1024 queries, assigned zig-zag
over 256-row blocks (h=0: 0,2,5,7; h=1: 1,3,4,6) with per-chunk KV prefixes
512/1024/1536/2048 (max over the pair). Exact causality via host-precomputed
additive masks (0 / -1e30) added to score PSUM before exp.

QKV projections run on the TensorEngine in fp8-e4m3 DoubleRow mode (2 k-tiles
per matmul, 0.5 cyc/row) with a 3-term residual expansion per operand pair:
x ~= xh + xl, W ~= Wh + Wl (each fp8), and x@W ~= xh@Wh + xl@Wh + xh@Wl.
That is 1.5x the fp8 matmul volume = 0.75x bf16 cycles, with ~bf16 accuracy
(hi/lo together carry ~7 mantissa bits). W is pre-scaled by 32 on the host so
its entries are O(1) (raw W ~ N(0,1/1024) would hit e4m3's subnormal floor);
the exp scale absorbs 32*32 and the softmax denominator absorbs the last 32
via a 32-valued ones column. Loops are ordered stationary-major (each fp8
weight pair feeds 2-4 moving matmuls) so the doubled 256-column LDWEIGHTS
stays hidden behind compute. Attention (scores, exp, AV) stays bf16 exactly
as before, with f32 PSUM accumulation throughout.

Hardware-wait notes: walrus accepts a single sync wait per engine instruction
(bacc's generate_event_semaphores legalizes the rest, at a cost). To keep the
sem chains short: every DMA lands in a fresh/stable region and is "blessed" by
an in-place DVE copy (as uint32 over the fp8/bf16 bytes, 4 bytes/el) so
matmul dependencies collapse onto the DVE semaphore; the exp (ACT) output is
DVE-copied into P^T for the same reason; PSUM lives in one pool (tags
big/rs = 6+2 = 8 banks) so slot WARs stay on DVE/ACT.

The `reps` parameter repeats the whole body inside the NEFF; test.py uses the
1x-vs-9x wall-clock slope to estimate per-execution device time.
"""

import numpy as np
import ml_dtypes

import concourse.bass as bass
from concourse import bacc
import concourse.mybir as mybir
from concourse.tile import TileContext
from concourse.bass_utils import run_bass_kernel_spmd

P = 128
B = 4
S = 2048          # sequence length (= keys per batch)
D = 1024          # d_in = d_out
HALF = 1024       # queries per core
CHUNK = 256       # query chunk
CD = D // P       # 8 contraction tiles
SK = S // P       # 16 key tiles
F = 512           # matmul moving free dim (one PSUM bank of f32)
KV = (512, 1024, 1536, 2048)   # kv prefix length per chunk
QBASE = (0, 256, 512, 768)     # local query offset of each chunk
WSCALE = 32.0         # host-side premultiplier on W (fp8 range)
SCALE = 1.0 / 32768.0  # exp scale: 1/(sqrt(d_k) * WSCALE^2)
NEG = -1.0e30         # additive mask for disallowed (k, q)

# global query-row block starts per half (zig-zag over 256-blocks)
QROWS = ((0, 512, 1280, 1792), (256, 768, 1024, 1536))

BF16 = ml_dtypes.bfloat16
E4 = ml_dtypes.float8_e4m3
DR = mybir.MatmulPerfMode.DoubleRow


def build_nc(reps: int = 1) -> bacc.Bacc:
    nc = bacc.Bacc("TRN2")
    bf = mybir.dt.bfloat16
    f8 = mybir.dt.float8e4
    f32 = mybir.dt.float32
    u32 = mybir.dt.uint32

    xkvh_d = nc.declare_dram_parameter("xkvh", [D, S], f8, isOutput=False)
    xkvl_d = nc.declare_dram_parameter("xkvl", [D, S], f8, isOutput=False)
    xqh_d = nc.declare_dram_parameter("xqh", [D, HALF], f8, isOutput=False)
    xql_d = nc.declare_dram_parameter("xql", [D, HALF], f8, isOutput=False)
    w_d = {
        (w, p): nc.declare_dram_parameter(f"w{w}{p}", [D, D], f8, isOutput=False)
        for w in "qkv" for p in "hl"
    }
    m_d = [
        nc.declare_dram_parameter(f"mask{ci}", [KV[ci], CHUNK], bf, isOutput=False)
        for ci in range(len(KV))
    ]
    out_d = nc.declare_dram_parameter("out", [HALF, D], bf, isOutput=True)

    with TileContext(nc) as tc:
        with tc.tile_pool(name="persist", bufs=1) as persist, \
             tc.tile_pool(name="work", bufs=1) as work, \
             tc.tile_pool(name="psum", bufs=1, space="PSUM") as psum:
            # K^T[d, k], V[k, d], Q^T[d, q] resident in SBUF (bf16)
            KT = persist.tile([P, CD, S], bf)
            Vt = persist.tile([P, SK, D], bf)
            QT = persist.tile([P, CD, HALF], bf)
            ones = persist.tile([P, 1], bf)
            nc.vector.memset(ones[:], WSCALE)

            # DMA into fresh regions, then "bless" with an in-place DVE copy
            # (uint32 view: 4 bytes per element) so consumers wait on DVE only.
            def load(dst, dram, c):
                nc.sync.dma_start(out=dst[:, c], in_=dram[c * P:(c + 1) * P, :])
                v = dst[:, c].bitcast(u32)
                nc.vector.tensor_copy(v, v)

            for _rep in range(reps):
                xkvh = work.tile([P, CD, S], f8, tag="bigA")
                xkvl = work.tile([P, CD, S], f8, tag="bigB")
                xqh = work.tile([P, CD, HALF], f8, tag="xqh")
                xql = work.tile([P, CD, HALF], f8, tag="xql")
                ws = {
                    (w, p): work.tile([P, CD, D], f8, tag=f"w{w}{p}",
                                      name=f"w{w}{p}_s")
                    for w in "qkv" for p in "hl"
                }
                # wk/xkv first so K^T matmuls can start as soon as data lands
                for c in range(CD):
                    load(ws[("k", "h")], w_d[("k", "h")], c)
                    load(xkvh, xkvh_d, c)
                for c in range(CD):
                    load(ws[("k", "l")], w_d[("k", "l")], c)
                    load(xkvl, xkvl_d, c)
                for c in range(CD):
                    load(ws[("v", "h")], w_d[("v", "h")], c)
                    load(ws[("v", "l")], w_d[("v", "l")], c)
                for c in range(CD):
                    load(ws[("q", "h")], w_d[("q", "h")], c)
                    load(xqh, xqh_d, c)
                for c in range(CD):
                    load(ws[("q", "l")], w_d[("q", "l")], c)
                    load(xql, xql_d, c)

                # ---------------- phase 1: QKV projections (fp8 DoubleRow) --
                # 3-term residual: out = xh@Wh + xl@Wh + xh@Wl, each matmul
                # contracting a pair of 128-deep k-tiles ([:, cp:cp+2, ...]).
                # Steps are stationary-major so each weight pair is reused
                # across 2 or 4 moving matmuls before the next LDWEIGHTS.

                # K^T[m, k] and Q^T[m, q]: stationary = W pair, moving = x.
                def proj_wx(wh, wl, xh, xl, nwin, out_tile, wlim=None):
                    # out_tile[:, m, win*F:(win+1)*F] = (x @ W)[mP:(m+1)P].T
                    for m in range(CD):
                        for g in range(nwin // 2):
                            ps = [psum.tile([P, F], f32, tag="big", bufs=6,
                                            name=f"pp{j}")
                                  for j in range(2)]
                            step = 0
                            for cp in range(0, CD, 2):
                                for st_t, mv_t in ((wh, xh), (wh, xl), (wl, xh)):
                                    for j in range(2):
                                        kf = 2 * g + j
                                        nc.tensor.matmul(
                                            ps[j][:],
                                            st_t[:, cp:cp + 2, m * P:(m + 1) * P],
                                            mv_t[:, cp:cp + 2, kf * F:(kf + 1) * F],
                                            start=(step == 0), stop=(step == 11),
                                            perf_mode=DR,
                                        )
                                    step += 1
                            for j in range(2):
                                kf = 2 * g + j
                                nc.vector.tensor_copy(
                                    out_tile[:, m, kf * F:(kf + 1) * F], ps[j][:])

                proj_wx(ws[("k", "h")], ws[("k", "l")], xkvh, xkvl, S // F, KT)
                proj_wx(ws[("q", "h")], ws[("q", "l")], xqh, xql, HALF // F, QT)

                # V[s, n]: stationary = x pair, moving = Wv.
                for st in range(SK):
                    ps = [psum.tile([P, F], f32, tag="big", bufs=6,
                                    name=f"pv{nf}")
                          for nf in range(2)]
                    step = 0
                    for cp in range(0, CD, 2):
                        for st_t, mv_t in ((xkvh, ws[("v", "h")]),
                                           (xkvh, ws[("v", "l")]),
                                           (xkvl, ws[("v", "h")])):
                            for nf in range(2):
                                nc.tensor.matmul(
                                    ps[nf][:],
                                    st_t[:, cp:cp + 2, st * P:(st + 1) * P],
                                    mv_t[:, cp:cp + 2, nf * F:(nf + 1) * F],
                                    start=(step == 0), stop=(step == 11),
                                    perf_mode=DR,
                                )
                            step += 1
                    for nf in range(2):
                        nc.vector.tensor_copy(Vt[:, st, nf * F:(nf + 1) * F],
                                              ps[nf][:])

                # ---------------- phase 2: attention (bf16) ----------------
                for ci in range(len(KV)):
                    nk = KV[ci] // P
                    qb = QBASE[ci]
                    md = m_d[ci]
                    # P^T = exp((S^T + mask)*SCALE), bf16, reuses the xkvh slot
                    PT = work.tile([P, SK, CHUNK], bf, tag="bigA")
                    # tiles whose last key row precedes both cores' first
                    # query row need no mask at all (additive mask is all 0)
                    vmin = min(QROWS[0][ci], QROWS[1][ci])
                    for ki in range(nk):
                        masked = ki * P + P - 1 > vmin
                        if masked:
                            # just-in-time mask tile, blessed onto DVE
                            mt = work.tile([P, CHUNK], bf, tag="mask", bufs=4)
                            nc.sync.dma_start(
                                out=mt[:], in_=md[ki * P:(ki + 1) * P, :])
                            mv = mt[:].bitcast(mybir.dt.uint32)
                            nc.vector.tensor_copy(mv, mv)
                        ps = psum.tile([P, CHUNK], f32, tag="big", bufs=6)
                        for c in range(CD):
                            nc.tensor.matmul(
                                ps[:],
                                KT[:, c, ki * P:(ki + 1) * P],
                                QT[:, c, qb:qb + CHUNK],
                                start=(c == 0), stop=(c == CD - 1),
                            )
                        if masked:
                            nc.vector.tensor_add(ps[:], ps[:], mt[:])
                        pe = work.tile([P, CHUNK], bf, tag="pexp", bufs=2)
                        nc.scalar.activation(
                            pe[:], ps[:],
                            mybir.ActivationFunctionType.Exp, scale=SCALE,
                        )
                        nc.vector.tensor_copy(PT[:, ki], pe[:])
                    for qj in range(CHUNK // P):
                        o0 = psum.tile([P, F], f32, tag="big", bufs=6)
                        o1 = psum.tile([P, F], f32, tag="big", bufs=6)
                        rs = psum.tile([P, 1], f32, tag="rs", bufs=2)
                        for ki in range(nk):
                            lh = PT[:, ki, qj * P:(qj + 1) * P]
                            st_, sp_ = (ki == 0), (ki == nk - 1)
                            nc.tensor.matmul(o0[:], lh, Vt[:, ki, 0:F],
                                             start=st_, stop=sp_)
                            nc.tensor.matmul(o1[:], lh, Vt[:, ki, F:2 * F],
                                             start=st_, stop=sp_)
                            nc.tensor.matmul(rs[:], lh, ones[:, 0:1],
                                             start=st_, stop=sp_)
                        rcp = work.tile([P, 1], f32, tag="rcp", bufs=4)
                        nc.vector.reciprocal(rcp[:], rs[:])
                        ot = work.tile([P, D], bf, tag="ot", bufs=4)
                        nc.vector.tensor_scalar_mul(ot[:, 0:F], o0[:], rcp[:])
                        nc.vector.tensor_scalar_mul(ot[:, F:2 * F], o1[:], rcp[:])
                        row = qb + qj * P
                        nc.sync.dma_start(out=out_d[row:row + P, :], in_=ot[:])
    nc.finalize()  # run bacc legalization (wait splitting, reg alloc)
    return nc


_NC_CACHE = {}


def _get_nc(reps: int = 1):
    if reps not in _NC_CACHE:
        _NC_CACHE[reps] = build_nc(reps)
    return _NC_CACHE[reps]


def _masks():
    """Additive bf16 masks per half: 0 where k <= global q position, else -1e30."""
    q = np.arange(CHUNK)[None, :]
    out = []
    for h in range(2):
        ms = []
        for ci in range(len(KV)):
            k = np.arange(KV[ci])[:, None]
            ms.append(np.where(k <= q + QROWS[h][ci], 0.0, NEG).astype(BF16))
        out.append(ms)
    return out


def _split8(a):
    hi = a.astype(E4)
    lo = (a - hi.astype(np.float32)).astype(E4)
    return hi, lo


def make_in_maps(x, Wq, Wk, Wv):
    w8 = {}
    for name, W in (("q", Wq), ("k", Wk), ("v", Wv)):
        hi, lo = _split8(np.ascontiguousarray(W, dtype=np.float32) * WSCALE)
        w8[name] = (hi, lo)
    masks = _masks()
    xb8 = []
    for b in range(B):
        xT = np.ascontiguousarray(np.asarray(x)[b].T, dtype=np.float32)
        xb8.append(_split8(xT))
    in_maps = []
    for i in range(8):
        b, h = i // 2, i % 2
        xh, xl = xb8[b]
        xqh = np.ascontiguousarray(
            np.concatenate([xh[:, r:r + CHUNK] for r in QROWS[h]], axis=1))
        xql = np.ascontiguousarray(
            np.concatenate([xl[:, r:r + CHUNK] for r in QROWS[h]], axis=1))
        m = {"xkvh": xh, "xkvl": xl, "xqh": xqh, "xql": xql}
        for w in "qkv":
            m[f"w{w}h"], m[f"w{w}l"] = w8[w]
        for ci in range(len(KV)):
            m[f"mask{ci}"] = masks[h][ci]
        in_maps.append(m)
    return in_maps


def gather_out(results, x_dtype=np.float32):
    out = np.empty((B, S, D), x_dtype)
    for i in range(8):
        b, h = i // 2, i % 2
        o = np.asarray(results[i]["out"]).astype(x_dtype)
        for ci, r in enumerate(QROWS[h]):
            out[b, r:r + CHUNK] = o[ci * CHUNK:(ci + 1) * CHUNK]
    return out


def run_cores(in_maps, **kwargs):
    return run_bass_kernel_spmd(_get_nc(), in_maps, core_ids=list(range(8)), **kwargs)


def kernel(x, Wq, Wk, Wv):
    x = np.asarray(x)
    in_maps = make_in_maps(x, np.asarray(Wq), np.asarray(Wk), np.asarray(Wv))
    res = run_cores(in_maps)
    return gather_out(res.results)


# revision 6
# speedup vs baseline: 1.4081x; 1.4081x over previous
"""Causal single-head attention (B=4, S=2048, d=1024, f32) on 8 TRN2 NeuronCores.

Sharding: core i = (batch b = i//2, half h = i%2). Queries are assigned
zig-zag over 256-row blocks (h=0 gets 0,2,5,7; h=1 gets 1,3,4,6) and each
core processes four 256-query chunks against KV prefixes of 512/1024/1536/
2048 keys. Exact causality via host-precomputed additive masks (0 / -1e30)
added to score PSUM before exp.

K/V projections are deduplicated across the two cores of a batch: each core
computes K^T and V only for the 256-row blocks of its own parity (h=0 even
blocks, h=1 odd), then the pair exchanges halves with an HBM AllGather over
replica groups [[0,1],[2,3],[4,5],[6,7]]. SPMD uniformity is preserved by
data placement: the host hands each core x^T with ITS parity's columns
packed into [0:1024), so the (identical) program always projects columns
[0:1024) and always scatters AllGather slot s into the true positions of
parity s. This cuts per-core projection work from 5.4 to 3.2 GMAC; the
exchange (2 MB out, 4 MB back per tensor) overlaps with the V/Q projections
on the PE.

Compute is bf16 on the TensorEngine with f32 PSUM accumulation; matmuls are
[c=128, m=128, n<=512]. Scores are computed transposed (S^T[k, q]) so P^T =
exp(S^T) feeds the AV matmul directly as lhsT, with the softmax denominator
from a ones-column matmul and the division folded into the PSUM->SBUF copy
of the output. No max-subtraction: scaled logits are bounded for these
inputs.

Hardware-wait notes: walrus accepts a single sync wait per engine
instruction (bacc legalizes the rest, at a cost). Every DMA lands in a
fresh/stable region and is "blessed" by an in-place DVE copy (uint32 view)
so matmul dependencies collapse onto the DVE semaphore; exp output is
DVE-copied into P^T for the same reason; PSUM lives in one pool (tags
big/rs = 6+2 = 8 banks) so slot WARs stay on DVE/ACT.

The `reps` parameter repeats the whole body inside the NEFF; test.py uses
the 1x-vs-9x wall-clock slope to estimate per-execution device time.
"""

import numpy as np
import ml_dtypes

import concourse.bass as bass
from concourse import bacc
import concourse.mybir as mybir
from concourse.tile import TileContext
from concourse.bass_utils import run_bass_kernel_spmd

P = 128
B = 4
S = 2048          # sequence length (= keys per batch)
D = 1024          # d_in = d_out
HALF = 1024       # queries per core; also K/V rows computed per core
CHUNK = 256       # query chunk
CD = D // P       # 8 contraction tiles
SK = S // P       # 16 key tiles
F = 512           # matmul moving free dim (one PSUM bank of f32)
KV = (512, 1024, 1536, 2048)   # kv prefix length per chunk
QBASE = (0, 256, 512, 768)     # local query offset of each chunk
SCALE = 1.0 / 32.0    # 1/sqrt(d_k)
NEG = -1.0e30         # additive mask for disallowed (k, q)

# global query-row block starts per half (zig-zag over 256-blocks)
QROWS = ((0, 512, 1280, 1792), (256, 768, 1024, 1536))

GROUPS = [[0, 1], [2, 3], [4, 5], [6, 7]]

BF16 = ml_dtypes.bfloat16


def build_nc(reps: int = 1) -> bacc.Bacc:
    nc = bacc.Bacc("TRN2")
    bf = mybir.dt.bfloat16
    f32 = mybir.dt.float32
    u32 = mybir.dt.uint32

    # x^T with this core's parity columns packed into [0:1024)
    xkv_d = nc.declare_dram_parameter("xkv", [D, HALF], bf, isOutput=False)
    xq_d = nc.declare_dram_parameter("xq", [D, HALF], bf, isOutput=False)
    wq_d = nc.declare_dram_parameter("wq", [D, D], bf, isOutput=False)
    wk_d = nc.declare_dram_parameter("wk", [D, D], bf, isOutput=False)
    wv_d = nc.declare_dram_parameter("wv", [D, D], bf, isOutput=False)
    m_d = [
        nc.declare_dram_parameter(f"mask{ci}", [KV[ci], CHUNK], bf, isOutput=False)
        for ci in range(len(KV))
    ]
    out_d = nc.declare_dram_parameter("out", [HALF, D], bf, isOutput=True)

    with TileContext(nc) as tc:
        with tc.tile_pool(name="persist", bufs=1) as persist, \
             tc.tile_pool(name="work", bufs=1) as work, \
             tc.tile_pool(name="dram", bufs=1, space="DRAM") as dram, \
             tc.tile_pool(name="psum", bufs=1, space="PSUM") as psum:
            # K^T[d, k] as [p, c, jj, 512] (true col = 512*jj + inner);
            # V[k, d] as [p, j, t, d] (true s-tile = 4*j + t);
            # Q^T[d, q] resident in SBUF (bf16)
            KT = persist.tile([P, CD, 4, F], bf)
            Vt = persist.tile([P, 4, 4, D], bf)
            QT = persist.tile([P, CD, HALF], bf)
            ones = persist.tile([P, 1], bf)
            nc.vector.memset(ones[:], 1.0)

            def load(dst, dram_t, c):
                nc.sync.dma_start(out=dst[:, c], in_=dram_t[c * P:(c + 1) * P, :])
                v = dst[:, c].bitcast(u32)
                nc.vector.tensor_copy(v, v)

            for _rep in range(reps):
                # exchange bounce buffers (fresh per rep)
                kx_in = dram.tile([CD, P, HALF], bf, tag="kxi", name="kx_in")
                kx_out = dram.tile([2, CD, P, 4, CHUNK], bf, tag="kxo",
                                   name="kx_out")
                v_in = dram.tile([8, P, D], bf, tag="vxi", name="v_in")
                v_out = dram.tile([2, 8, P, D], bf, tag="vxo", name="v_out")

                xkv_s = work.tile([P, CD, HALF], bf, tag="xkv")
                xq_s = work.tile([P, CD, HALF], bf, tag="xq")
                wq_s = work.tile([P, CD, D], bf, tag="wq")
                wk_s = work.tile([P, CD, D], bf, tag="wk")
                wv_s = work.tile([P, CD, D], bf, tag="wv")
                for c in range(CD):
                    load(wk_s, wk_d, c)
                    load(xkv_s, xkv_d, c)
                for c in range(CD):
                    load(wv_s, wv_d, c)
                    load(wq_s, wq_d, c)
                    load(xq_s, xq_d, c)

                # ---------------- phase 1: QKV projections ----------------
                # K^T[m, k] for my 1024 columns; staged per-m and sent to the
                # pair AllGather.
                for m in range(CD):
                    kst = work.tile([P, HALF], bf, tag="kst", bufs=2, name="kst")
                    for kf in range(HALF // F):
                        ps = psum.tile([P, F], f32, tag="big", bufs=6, name="pp")
                        for c in range(CD):
                            nc.tensor.matmul(
                                ps[:],
                                wk_s[:, c, m * P:(m + 1) * P],
                                xkv_s[:, c, kf * F:(kf + 1) * F],
                                start=(c == 0), stop=(c == CD - 1),
                            )
                        nc.vector.tensor_copy(kst[:, kf * F:(kf + 1) * F], ps[:])
                    nc.sync.dma_start(out=kx_in[m], in_=kst[:])
                nc.gpsimd.collective_compute(
                    "AllGather", mybir.AluOpType.bypass,
                    replica_groups=GROUPS,
                    ins=[kx_in[:].opt()], outs=[kx_out[:].opt()],
                )
                # V[s, n] for my 8 s-tiles
                for st in range(8):
                    vst = work.tile([P, D], bf, tag="vst", bufs=2, name="vst")
                    for nf in range(D // F):
                        ps = psum.tile([P, F], f32, tag="big", bufs=6, name="pv")
                        for c in range(CD):
                            nc.tensor.matmul(
                                ps[:],
                                xkv_s[:, c, st * P:(st + 1) * P],
                                wv_s[:, c, nf * F:(nf + 1) * F],
                                start=(c == 0), stop=(c == CD - 1),
                            )
                        nc.vector.tensor_copy(vst[:, nf * F:(nf + 1) * F], ps[:])
                    nc.sync.dma_start(out=v_in[st], in_=vst[:])
                nc.gpsimd.collective_compute(
                    "AllGather", mybir.AluOpType.bypass,
                    replica_groups=GROUPS,
                    ins=[v_in[:].opt()], outs=[v_out[:].opt()],
                )
                # K readback: slot par holds parity-par strips; true position
                # of strip j is col 512*j + 256*par.
                for m in range(CD):
                    for par in range(2):
                        dst = KT[:, m, :, par * CHUNK:(par + 1) * CHUNK]
                        nc.sync.dma_start(out=dst, in_=kx_out[par, m])
                        v = dst.bitcast(u32)
                        nc.vector.tensor_copy(v, v)
                # Q^T[m, q]
                for m in range(CD):
                    for qf in range(HALF // F):
                        ps = psum.tile([P, F], f32, tag="big", bufs=6, name="pq")
                        for c in range(CD):
                            nc.tensor.matmul(
                                ps[:],
                                wq_s[:, c, m * P:(m + 1) * P],
                                xq_s[:, c, qf * F:(qf + 1) * F],
                                start=(c == 0), stop=(c == CD - 1),
                            )
                        nc.vector.tensor_copy(QT[:, m, qf * F:(qf + 1) * F], ps[:])
                # V readback: sender tile st' = 2j+t of slot par is true
                # s-tile 4j + 2par + t.
                for par in range(2):
                    for stp in range(8):
                        j, t = stp // 2, stp % 2
                        dst = Vt[:, j, 2 * par + t, :]
                        nc.sync.dma_start(out=dst, in_=v_out[par, stp])
                        v = dst.bitcast(u32)
                        nc.vector.tensor_copy(v, v)

                # ---------------- phase 2: attention ----------------
                for ci in range(len(KV)):
                    nk = KV[ci] // P
                    qb = QBASE[ci]
                    md = m_d[ci]
                    # P^T = exp((S^T + mask)/32), bf16, reuses the xkv slot
                    PT = work.tile([P, SK, CHUNK], bf, tag="xkv")
                    vmin = min(QROWS[0][ci], QROWS[1][ci])
                    for ki in range(nk):
                        masked = ki * P + P - 1 > vmin
                        if masked:
                            mt = work.tile([P, CHUNK], bf, tag="mask", bufs=4,
                                           name="mt")
                            nc.sync.dma_start(
                                out=mt[:], in_=md[ki * P:(ki + 1) * P, :])
                            mv = mt[:].bitcast(u32)
                            nc.vector.tensor_copy(mv, mv)
                        ps = psum.tile([P, CHUNK], f32, tag="big", bufs=6,
                                       name="psc")
                        for c in range(CD):
                            nc.tensor.matmul(
                                ps[:],
                                KT[:, c, ki // 4, (ki % 4) * P:(ki % 4 + 1) * P],
                                QT[:, c, qb:qb + CHUNK],
                                start=(c == 0), stop=(c == CD - 1),
                            )
                        if masked:
                            nc.vector.tensor_add(ps[:], ps[:], mt[:])
                        pe = work.tile([P, CHUNK], bf, tag="pexp", bufs=2,
                                       name="pe")
                        nc.scalar.activation(
                            pe[:], ps[:],
                            mybir.ActivationFunctionType.Exp, scale=SCALE,
                        )
                        nc.vector.tensor_copy(PT[:, ki], pe[:])
                    for qj in range(CHUNK // P):
                        o0 = psum.tile([P, F], f32, tag="big", bufs=6, name="o0")
                        o1 = psum.tile([P, F], f32, tag="big", bufs=6, name="o1")
                        rs = psum.tile([P, 1], f32, tag="rs", bufs=2, name="rs")
                        for ki in range(nk):
                            lh = PT[:, ki, qj * P:(qj + 1) * P]
                            st_, sp_ = (ki == 0), (ki == nk - 1)
                            nc.tensor.matmul(o0[:], lh,
                                             Vt[:, ki // 4, ki % 4, 0:F],
                                             start=st_, stop=sp_)
                            nc.tensor.matmul(o1[:], lh,
                                             Vt[:, ki // 4, ki % 4, F:2 * F],
                                             start=st_, stop=sp_)
                            nc.tensor.matmul(rs[:], lh, ones[:, 0:1],
                                             start=st_, stop=sp_)
                        rcp = work.tile([P, 1], f32, tag="rcp", bufs=4,
                                        name="rcp")
                        nc.vector.reciprocal(rcp[:], rs[:])
                        ot = work.tile([P, D], bf, tag="ot", bufs=4, name="ot")
                        nc.vector.tensor_scalar_mul(ot[:, 0:F], o0[:], rcp[:])
                        nc.vector.tensor_scalar_mul(ot[:, F:2 * F], o1[:], rcp[:])
                        row = qb + qj * P
                        nc.sync.dma_start(out=out_d[row:row + P, :], in_=ot[:])
    nc.finalize()  # run bacc legalization (wait splitting, reg alloc)
    return nc


_NC_CACHE = {}


def _get_nc(reps: int = 1):
    if reps not in _NC_CACHE:
        _NC_CACHE[reps] = build_nc(reps)
    return _NC_CACHE[reps]


def _masks():
    """Additive bf16 masks per half: 0 where k <= global q position, else -1e30."""
    q = np.arange(CHUNK)[None, :]
    out = []
    for h in range(2):
        ms = []
        for ci in range(len(KV)):
            k = np.arange(KV[ci])[:, None]
            ms.append(np.where(k <= q + QROWS[h][ci], 0.0, NEG).astype(BF16))
        out.append(ms)
    return out


def make_in_maps(x, Wq, Wk, Wv):
    wqb = np.ascontiguousarray(np.asarray(Wq).astype(BF16))
    wkb = np.ascontiguousarray(np.asarray(Wk).astype(BF16))
    wvb = np.ascontiguousarray(np.asarray(Wv).astype(BF16))
    masks = _masks()
    in_maps = []
    xT = [np.ascontiguousarray(np.asarray(x)[b].T.astype(BF16)) for b in range(B)]
    for i in range(8):
        b, h = i // 2, i % 2
        # my parity's 256-col blocks packed into [0:1024)
        xkv = np.ascontiguousarray(np.concatenate(
            [xT[b][:, 512 * j + 256 * h: 512 * j + 256 * h + 256]
             for j in range(4)], axis=1))
        xq = np.ascontiguousarray(np.concatenate(
            [xT[b][:, r:r + CHUNK] for r in QROWS[h]], axis=1))
        m = {"xkv": xkv, "xq": xq, "wq": wqb, "wk": wkb, "wv": wvb}
        for ci in range(len(KV)):
            m[f"mask{ci}"] = masks[h][ci]
        in_maps.append(m)
    return in_maps


def gather_out(results, x_dtype=np.float32):
    out = np.empty((B, S, D), x_dtype)
    for i in range(8):
        b, h = i // 2, i % 2
        o = np.asarray(results[i]["out"]).astype(x_dtype)
        for ci, r in enumerate(QROWS[h]):
            out[b, r:r + CHUNK] = o[ci * CHUNK:(ci + 1) * CHUNK]
    return out


def run_cores(in_maps, **kwargs):
    return run_bass_kernel_spmd(_get_nc(), in_maps, core_ids=list(range(8)), **kwargs)


def kernel(x, Wq, Wk, Wv):
    x = np.asarray(x)
    in_maps = make_in_maps(x, np.asarray(Wq), np.asarray(Wk), np.asarray(Wv))
    res = run_cores(in_maps)
    return gather_out(res.results)
